# revision 1
# baseline (speedup 1.0000x reference)
"""Distributed GQA attention kernel for 8 Trainium2 NeuronCores.

Contract: kernel(**inputs) takes the FULL unsharded inputs of the reference
nn.Module (x, Wq, bq, Wk, bk, Wv, bv, Wo, bo) and returns the FULL
[B, T, E] float32 output.

Sharding: tensor-parallel over kv heads. Core c owns kv head c and q heads
4c..4c+3: it projects q/k/v for its heads over all tokens (softmax scale
pre-folded into Wq), applies RoPE on-chip, runs causal attention in a
transposed-score layout (S^T[k,q] so the exp output feeds the PV matmul with
no transpose; a ones-column appended to V yields softmax denominators; no
max-subtraction pass - logits are bounded by construction), reshards the
attention output head-major -> token-major with two half-size AllToAlls per
batch, and computes o_proj with the full Wo on a disjoint 512-token slice.
The host only slices/concatenates.
"""

from contextlib import ExitStack

import concourse.bass as bass
import concourse.mybir as mybir
import concourse.tile as tile
from concourse import bacc
from concourse.masks import make_identity

F32 = mybir.dt.float32
BF16 = mybir.dt.bfloat16
AF = mybir.ActivationFunctionType
ALU = mybir.AluOpType

N_CORES = 8


def build(B=2, T=2048, E=2048, D=64, HQ_PER_CORE=4, repeat=1, debug_taps=False,
          no_collective=False, split_a2a=True):
    BT = B * T
    DQ = HQ_PER_CORE * D          # 256
    TS = T // N_CORES             # per-core token slice per batch
    KC = E // 128                 # contraction chunks
    NB = BT // 512                # projection column chunks
    QC = T // 512                 # q chunks per batch
    NSPC = 512 // TS              # token slices per 512-col q chunk

    nc = bacc.Bacc("TRN2", target_bir_lowering=False, debug=False,
                   num_devices=N_CORES)

    xT = nc.dram_tensor("xT", [E, BT], BF16, kind="ExternalInput").ap()
    wqT = nc.dram_tensor("wqT", [E, DQ], BF16, kind="ExternalInput").ap()
    wkT = nc.dram_tensor("wkT", [E, D], BF16, kind="ExternalInput").ap()
    wvT = nc.dram_tensor("wvT", [E, D], BF16, kind="ExternalInput").ap()
    bq = nc.dram_tensor("bq", [DQ, 1], F32, kind="ExternalInput").ap()
    bkv = nc.dram_tensor("bkv", [128, 1], F32, kind="ExternalInput").ap()
    woT = nc.dram_tensor("woT", [E, E], BF16, kind="ExternalInput").ap()
    bo = nc.dram_tensor("bo", [1, E], BF16, kind="ExternalInput").ap()
    cos_d = nc.dram_tensor("cosf", [128, BT], BF16, kind="ExternalInput").ap()
    sin_d = nc.dram_tensor("sinm", [128, BT], BF16, kind="ExternalInput").ap()
    mask_d = nc.dram_tensor("mask", [128, 128], BF16, kind="ExternalInput").ap()
    out = nc.dram_tensor("out", [B * TS, E], F32, kind="ExternalOutput").ap()

    taps = {}
    if debug_taps:
        taps["qT"] = nc.dram_tensor("qT_dump", [128, (DQ // 128) * BT], BF16,
                                    kind="ExternalOutput").ap()
        taps["kT"] = nc.dram_tensor("kT_dump", [128, BT], BF16,
                                    kind="ExternalOutput").ap()
        taps["va"] = nc.dram_tensor("va_dump", [128, (BT // 128) * (D + 1)], BF16,
                                    kind="ExternalOutput").ap()
        taps["a2a_in0"] = nc.dram_tensor("a2a_in0_dump", [8 * DQ, TS], BF16,
                                         kind="ExternalOutput").ap()
        taps["a2a_out0"] = nc.dram_tensor("a2a_out0_dump", [8 * DQ, TS], BF16,
                                          kind="ExternalOutput").ap()
    args = dict(split_a2a=split_a2a, no_collective=no_collective, taps=taps, B=B, T=T, E=E, D=D, HQ=HQ_PER_CORE, BT=BT, DQ=DQ, TS=TS,
                KC=KC, NB=NB, QC=QC, NSPC=NSPC,
                xT=xT, wqT=wqT, wkT=wkT, wvT=wvT, bq=bq, bkv=bkv,
                woT=woT, bo=bo, cos_d=cos_d, sin_d=sin_d, mask_d=mask_d,
                out=out)
    with tile.TileContext(nc) as tc:
        for _ in range(repeat):
            _emit(tc, nc, args)
    nc.compile()
    return nc


def _emit(tc, nc, v):
    B, T, E, D, HQ = v["B"], v["T"], v["E"], v["D"], v["HQ"]
    BT, DQ, TS, KC, NB, QC, NSPC = (v["BT"], v["DQ"], v["TS"], v["KC"],
                                    v["NB"], v["QC"], v["NSPC"])
    xT, wqT, wkT, wvT, bq, bkv, woT, bo = (
        v["xT"], v["wqT"], v["wkT"], v["wvT"], v["bq"], v["bkv"],
        v["woT"], v["bo"])
    cos_d, sin_d, mask_d, out = v["cos_d"], v["sin_d"], v["mask_d"], v["out"]

    with ExitStack() as ctx:
        # ---- persistent SBUF ----
        pers = ctx.enter_context(tc.tile_pool(name="pers", bufs=1))
        wq_sb = pers.tile([128, KC, DQ], BF16, tag="wq")
        wkv_sb = pers.tile([128, KC, 2 * D], BF16, tag="wkv")
        bq_sb = pers.tile([128, DQ // 128], F32, tag="bq")
        bkv_sb = pers.tile([128, 1], F32, tag="bkv")
        wo_sb = pers.tile([128, KC, E], BF16, tag="wo")
        bo_sb = pers.tile([1, E], BF16, tag="bo")
        bo_bc = pers.tile([128, E], BF16, tag="bo_bc")
        mask_sb = pers.tile([128, 128], BF16, tag="mask")

        ident_sb = pers.tile([128, 128], BF16, tag="ident")
        qT_sb = pers.tile([128, HQ // 2, BT], BF16, tag="qT")
        kT_sb = pers.tile([128, BT], BF16, tag="kT")
        vaug_sb = pers.tile([128, BT // 128, D + 1], BF16, tag="vaug")

        nc.sync.dma_start(wkv_sb[:, :, 0:D], wkT.rearrange("(kc p) m -> p kc m", p=128))
        nc.sync.dma_start(wkv_sb[:, :, D:2 * D], wvT.rearrange("(kc p) m -> p kc m", p=128))
        nc.sync.dma_start(bkv_sb[:], bkv[:])
        make_identity(nc, ident_sb[:])
        nc.vector.memset(vaug_sb[:, :, D:D + 1], 1.0)

        dram = ctx.enter_context(tc.tile_pool(name="dram", bufs=1, space="DRAM"))
        nsp = 2 if v["split_a2a"] else 1
        rows_a2a = 8 * DQ // nsp
        a2a_in = [[dram.tile([rows_a2a, TS], BF16, name=f"a2a_in{b}_{hf}",
                             tag=f"a2a_in{b}_{hf}") for hf in range(nsp)]
                  for b in range(B)]
        a2a_out = [[dram.tile([rows_a2a, TS], BF16, name=f"a2a_out{b}_{hf}",
                              tag=f"a2a_out{b}_{hf}") for hf in range(nsp)]
                   for b in range(B)]

        # ---- phase 1a: k/v projections + rope + v transpose (all nb) ----
        with tc.tile_pool(name="p1asb", bufs=2) as p1asb, \
             tc.tile_pool(name="p1aps", bufs=2, space="PSUM") as p1aps:
            for nb in range(NB):
                ns = slice(nb * 512, (nb + 1) * 512)
                xt = p1asb.tile([128, KC, 512], BF16, tag="xt")
                kq4 = KC // 4
                for xi in range(4):
                    nc.sync.dma_start(
                        xt[:, xi * kq4:(xi + 1) * kq4],
                        xT[xi * kq4 * 128:(xi + 1) * kq4 * 128, ns]
                        .rearrange("(kc p) n -> p kc n", p=128))
                if nb == 0:
                    # q-projection weights: needed only from phase 1b on;
                    # issued behind the first x chunk so k/v start unblocked
                    nc.sync.dma_start(wq_sb[:], wqT.rearrange("(kc p) m -> p kc m", p=128))
                    nc.sync.dma_start(bq_sb[:], bq.rearrange("(mb p) o -> p (mb o)", p=128))
                    nc.sync.dma_start(mask_sb[:], mask_d[:])
                # k|v stacked in one stationary: one N=512 chain computes
                # both (out rows 0:64 = k, 64:128 = v) - halves the PE time
                # vs two M=64 chains that each idle half the array
                pkv = p1aps.tile([128, 512], F32, tag="pkv")
                pv = pkv[D:128]
                for kc in range(KC):
                    nc.tensor.matmul(pkv[:], wkv_sb[:, kc], xt[:, kc],
                                     start=(kc == 0), stop=(kc == KC - 1))
                cosa = p1asb.tile([D, 512], BF16, tag="cosa")
                sina = p1asb.tile([D, 512], BF16, tag="sina")
                nc.sync.dma_start(cosa[:], cos_d[0:D, ns])
                nc.sync.dma_start(sina[:], sin_d[0:D, ns])
                kvf = p1asb.tile([128, 512], BF16, tag="kvf")
                nc.scalar.activation(kvf[:], pkv[:], AF.Identity, bias=bkv_sb[:])
                # rope on k (rows 0:64)
                ksw = p1asb.tile([D, 512], BF16, tag="ksw")
                nc.sync.dma_start(ksw[0:32], kvf[32:64])
                nc.sync.dma_start(ksw[32:64], kvf[0:32])
                tk = p1asb.tile([D, 512], BF16, tag="tk")
                nc.vector.tensor_mul(tk[:], kvf[0:D], cosa[:])
                nc.vector.tensor_mul(ksw[:], ksw[:], sina[:])
                nc.vector.tensor_add(kT_sb[0:D, ns], tk[:], ksw[:])
                nc.sync.dma_start(kT_sb[D:128, ns], kT_sb[0:D, ns])
                # v (rows 64:128): transpose to token-major straight from kvf
                pvt = p1aps.tile([128, 4, D], BF16, tag="pvt")
                for i in range(4):
                    nc.tensor.transpose(pvt[:, i, :], kvf[D:128, i * 128:(i + 1) * 128],
                                        ident_sb[D:128, D:128])
                nc.any.tensor_copy(vaug_sb[:, nb * 4:nb * 4 + 4, 0:D], pvt[:])

        # ---- phases 1b/2/3, interleaved: q-proj chunk nb feeds attention
        # chunk (b0, qc=nb) immediately; A2A0 fires under b1's q-proj;
        # o_proj b0 runs before b1 attention on the PE.
        with ExitStack() as ctx2:
            p2sb = ctx2.enter_context(tc.tile_pool(name="p2sb", bufs=3))
            p2ps = ctx2.enter_context(tc.tile_pool(name="p2ps", bufs=2, space="PSUM"))
            p2acc = ctx2.enter_context(tc.tile_pool(name="p2acc", bufs=1, space="PSUM"))
            p3sb = ctx2.enter_context(tc.tile_pool(name="p3sb", bufs=2))
            p1bsb = ctx2.enter_context(tc.tile_pool(name="p1bsb", bufs=2))
            p1bps_cm = tc.tile_pool(name="p1bps", bufs=2, space="PSUM")
            p1bps = p1bps_cm.__enter__()

            def emit_stageB(nb):
                ns = slice(nb * 512, (nb + 1) * 512)
                xt = p1bsb.tile([128, KC, 512], BF16, tag="xt", name="xt")
                kq4 = KC // 4
                for xi in range(4):
                    nc.sync.dma_start(
                        xt[:, xi * kq4:(xi + 1) * kq4],
                        xT[xi * kq4 * 128:(xi + 1) * kq4 * 128, ns]
                        .rearrange("(kc p) n -> p kc n", p=128))
                cosb = p1bsb.tile([128, 512], BF16, tag="cosb", name="cosb")
                sinb = p1bsb.tile([128, 512], BF16, tag="sinb", name="sinb")
                nc.sync.dma_start(cosb[:], cos_d[:, ns])
                nc.sync.dma_start(sinb[:], sin_d[:, ns])
                for mb in range(DQ // 128):
                    pq = p1bps.tile([128, 512], F32, tag="pq", name="pq")
                    for kc in range(KC):
                        nc.tensor.matmul(pq[:], wq_sb[:, kc, mb * 128:(mb + 1) * 128],
                                         xt[:, kc], start=(kc == 0), stop=(kc == KC - 1))
                    qf = p1bsb.tile([128, 512], BF16, tag="qf", name="qf")
                    nc.vector.tensor_scalar_add(qf[:], pq[:], bq_sb[:, mb:mb + 1])
                    qsw = p1bsb.tile([128, 512], BF16, tag="qsw", name="qsw")
                    for g in range(2):
                        o = g * 64
                        nc.sync.dma_start(qsw[o:o + 32], qf[o + 32:o + 64])
                        nc.sync.dma_start(qsw[o + 32:o + 64], qf[o:o + 32])
                    tq = p1bsb.tile([128, 512], BF16, tag="tq", name="tq")
                    nc.vector.tensor_mul(tq[:], qf[:], cosb[:])
                    nc.vector.tensor_mul(qsw[:], qsw[:], sinb[:])
                    nc.vector.tensor_add(qT_sb[:, mb, ns], tq[:], qsw[:])

            def emit_attention(b, qc):
                n_kb = qc * 4 + 4
                for hp in range(HQ // 2):
                    yaccs = [p2acc.tile([D + 1, 512], F32, name=f"yacc{i}",
                                        tag=f"yacc{i}") for i in range(2)]
                    for kb in range(n_kb):
                        off = max(0, (kb - 4 * qc) * 128)
                        ncols = 512 - off
                        diag = kb >= 4 * qc
                        qcol = b * T + qc * 512 + off
                        st = p2ps.tile([128, 2, 512], F32, tag="st", name="st")
                        for i in range(2):
                            h = 2 * hp + i
                            po = (h % 2) * D
                            nc.tensor.matmul(
                                st[:, i, :ncols],
                                kT_sb[po:po + D, b * T + kb * 128: b * T + (kb + 1) * 128],
                                qT_sb[po:po + D, h // 2, qcol:qcol + ncols],
                                start=True, stop=True)
                        pt = p2sb.tile([128, 2, 512], BF16, tag="pt", name="pt", bufs=4)
                        nc.scalar.activation(pt[:, :, :ncols], st[:, :, :ncols], AF.Exp)
                        if diag:
                            for i in range(2):
                                nc.vector.tensor_mul(pt[:, i, 0:128], pt[:, i, 0:128],
                                                     mask_sb[:])
                        for i in range(2):
                            nc.tensor.matmul(
                                yaccs[i][:, off:512],
                                vaug_sb[:, b * (T // 128) + kb, :],
                                pt[:, i, :ncols],
                                start=(kb == 0), stop=(kb == n_kb - 1))
                    for i in range(2):
                        h = 2 * hp + i
                        # l-row (psum partition 64) -> partition 0 via aligned
                        # copy + SBUF-SBUF DMA; cross-base DVE reads are
                        # broken on HW (verified), DMA is the partition mover
                        lsc = p2sb.tile([D + 1, 512], F32, tag="lsc", bufs=2, name="lsc")
                        nc.vector.tensor_copy(lsc[D:D + 1, :], yaccs[i][D:D + 1, :])
                        lsb = p2sb.tile([1, 512], F32, tag="lsb", name="lsb")
                        nc.sync.dma_start(lsb[:], lsc[D:D + 1, :])
                        r = p2sb.tile([1, 512], F32, tag="r", name="r")
                        nc.vector.reciprocal_approx_fast(out=r[:], in_=lsb[:])
                        rb = p2sb.tile([D, 512], F32, tag="rb", bufs=2, name="rb")
                        nc.gpsimd.partition_broadcast(rb[:], r[:])
                        yt = p2sb.tile([D, 512], BF16, tag="yt", bufs=2, name="yt")
                        nc.vector.tensor_mul(yt[:], yaccs[i][0:D, :], rb[:])
                        for ss in range(NSPC):
                            j = qc * NSPC + ss
                            if v["split_a2a"]:
                                tgt, row = a2a_in[b][h // 2], j * 128 + (h % 2) * D
                            else:
                                tgt, row = a2a_in[b][0], j * DQ + h * D
                            nc.sync.dma_start(tgt[row: row + D, :],
                                              yt[:, ss * TS:(ss + 1) * TS])

            def emit_a2a(b, hf):
                if v["no_collective"]:
                    nc.sync.dma_start(a2a_out[b][hf][:], a2a_in[b][hf][:])
                else:
                    nc.gpsimd.collective_compute(
                        "AllToAll", ALU.bypass,
                        replica_groups=[list(range(N_CORES))],
                        ins=[a2a_in[b][hf].opt()], outs=[a2a_out[b][hf].opt()])

            def load_ya(b):
                # lo half first: its kc chunks (even) start accumulating while
                # the hi-half collective is still in flight
                yah = []
                for hf in range(nsp):
                    ya = p3sb.tile([128, KC // nsp, TS], BF16, tag=f"ya{b}_{hf}",
                                   name=f"ya{b}_{hf}", bufs=1)
                    src = a2a_out[b][hf].opt().rearrange("(kc p) t -> p kc t", p=128)
                    kq = KC // nsp // 4
                    for yi in range(4):
                        nc.sync.dma_start(ya[:, yi * kq:(yi + 1) * kq],
                                          src[:, yi * kq:(yi + 1) * kq])
                    yah.append(ya)
                return yah

            def emit_oproj(b, p3ps, yah):
                if v["split_a2a"]:
                    kc_order = ([2 * i for i in range(KC // 2)]
                                + [2 * i + 1 for i in range(KC // 2)])
                else:
                    kc_order = list(range(KC))
                for tb in range((TS + 127) // 128):
                    rows = min(128, TS - tb * 128)
                    for oc in range(E // 512):
                        ocs = slice(oc * 512, (oc + 1) * 512)
                        po = p3ps.tile([128, 512], F32, tag="po", name="po")
                        for ki, kc in enumerate(kc_order):
                            if v["split_a2a"]:
                                ya_ap = yah[kc % 2][:, kc // 2]
                            else:
                                ya_ap = yah[0][:, kc]
                            nc.tensor.matmul(po[:rows],
                                             ya_ap[:, tb * 128: tb * 128 + rows],
                                             wo_sb[:, kc, ocs], start=(ki == 0),
                                             stop=(ki == KC - 1))
                        osb = p3sb.tile([128, 512], F32, tag="osb", name="osb")
                        nc.vector.tensor_tensor(osb[:rows], po[:rows], bo_bc[:rows, ocs],
                                                ALU.add)
                        nc.sync.dma_start(
                            out[b * TS + tb * 128: b * TS + tb * 128 + rows, ocs],
                            osb[:rows])

            for nb in range(QC):
                emit_stageB(nb)
                emit_attention(0, nb)
            for hf in range(nsp):
                emit_a2a(0, hf)
            nc.sync.dma_start(bo_sb[:], bo[:])
            nc.gpsimd.partition_broadcast(bo_bc[:], bo_sb[:])
            # o_proj weights split per output-chunk, interleaved between the
            # b1 q-projection chunks so neither starves the other's DMAs
            woT_r = woT.rearrange("(kc p) m -> p kc m", p=128)
            wo_chunk = E // (NB - QC)
            for nb in range(QC, NB):
                emit_stageB(nb)
                o0 = (nb - QC) * wo_chunk
                nc.sync.dma_start(wo_sb[:, :, o0:o0 + wo_chunk],
                                  woT_r[:, :, o0:o0 + wo_chunk])
            p1bps_cm.__exit__(None, None, None)
            p3ps = ctx2.enter_context(tc.tile_pool(name="p3ps", bufs=2, space="PSUM"))
            yah0 = None
            for qc in range(QC):
                emit_attention(1, qc)
                if qc == 0:
                    # b0's o_proj operand: loaded mid-b1-attention when the
                    # sync DMA queue is quiet (data final since A2A0)
                    yah0 = load_ya(0)
            for hf in range(nsp):
                emit_a2a(1, hf)
            # o_proj b0 placed here so its PE work covers the A2A1 window
            emit_oproj(0, p3ps, yah0)
            emit_oproj(1, p3ps, load_ya(1))

            taps = v["taps"]
            if taps:
                nc.sync.dma_start(taps["qT"][:], qT_sb.rearrange("p m t -> p (m t)"))
                nc.sync.dma_start(taps["kT"][:], kT_sb[:])
                nc.sync.dma_start(taps["va"][:], vaug_sb.rearrange("p b d -> p (b d)"))
                nc.sync.dma_start(taps["a2a_in0"][:, 0:TS], a2a_in[0][0][:, 0:TS])
                nc.sync.dma_start(taps["a2a_out0"][:, 0:TS], a2a_out[0][0][:, 0:TS])


# ---------------------------------------------------------------------------
# host-side sharding, execution, and gather
# ---------------------------------------------------------------------------
import numpy as np
import ml_dtypes

BF = ml_dtypes.bfloat16
ROPE_BASE = 10000.0
_CACHE = {}


def _rope_tables(T, D):
    inv_freq = 1.0 / (ROPE_BASE ** (np.arange(0, D, 2, dtype=np.float64) / D))
    t = np.arange(T, dtype=np.float64)
    freqs = np.einsum("i,j->ij", t, inv_freq)
    return np.cos(freqs), np.sin(freqs)


def _make_core_inputs(x, Wq, bq, Wk, bk, Wv, bv, Wo, bo):
    B, T, E = x.shape
    D = 64
    DQ = Wq.shape[0] // N_CORES
    BT = B * T
    scale = 1.0 / np.sqrt(D)

    xT = np.ascontiguousarray(x.reshape(BT, E).T).astype(BF)
    woT = np.ascontiguousarray(Wo.T).astype(BF)
    bo_row = bo.reshape(1, E).astype(BF)

    cos, sin = _rope_tables(T, D)
    cos32 = np.tile(cos.T, (1, B))
    sin32 = np.tile(sin.T, (1, B))
    cosf = np.tile(cos32, (4, 1)).astype(BF)
    sgn = np.where((np.arange(128) % 64) < 32, -1.0, 1.0)[:, None]
    sinm = (np.tile(sin32, (4, 1)) * sgn).astype(BF)

    k_idx, q_idx = np.meshgrid(np.arange(128), np.arange(128), indexing="ij")
    mask = (q_idx >= k_idx).astype(BF)

    maps = []
    for c in range(N_CORES):
        qs = slice(c * DQ, (c + 1) * DQ)
        ks = slice(c * D, (c + 1) * D)
        maps.append({
            "xT": xT,
            "wqT": np.ascontiguousarray((Wq[qs] * scale).T).astype(BF),
            "wkT": np.ascontiguousarray(Wk[ks].T).astype(BF),
            "wvT": np.ascontiguousarray(Wv[ks].T).astype(BF),
            "bq": (bq[qs] * scale).reshape(DQ, 1).astype(np.float32),
            "bkv": np.concatenate([bk[ks], bv[ks]]).reshape(128, 1).astype(np.float32),
            "woT": woT,
            "bo": bo_row,
            "cosf": cosf,
            "sinm": sinm,
            "mask": mask,
        })
    return maps


def kernel(x, Wq, bq, Wk, bk, Wv, bv, Wo, bo):
    from concourse import bass_utils

    x = np.asarray(x, dtype=np.float32)
    Wq, bq = np.asarray(Wq, np.float32), np.asarray(bq, np.float32)
    Wk, bk = np.asarray(Wk, np.float32), np.asarray(bk, np.float32)
    Wv, bv = np.asarray(Wv, np.float32), np.asarray(bv, np.float32)
    Wo, bo = np.asarray(Wo, np.float32), np.asarray(bo, np.float32)
    B, T, E = x.shape

    key = (B, T, E)
    if key not in _CACHE:
        _CACHE[key] = build(B=B, T=T, E=E)
    nc = _CACHE[key]

    maps = _make_core_inputs(x, Wq, bq, Wk, bk, Wv, bv, Wo, bo)
    res = bass_utils.run_bass_kernel_spmd(
        nc, maps, core_ids=list(range(N_CORES)))

    TS = T // N_CORES
    full = np.empty((B, T, E), dtype=np.float32)
    for c in range(N_CORES):
        o = res.results[c]["out"]
        for b in range(B):
            full[b, c * TS:(c + 1) * TS] = o[b * TS:(b + 1) * TS]
    return full



# revision 21
# speedup vs baseline: 1.0178x; 1.0178x over previous
"""Distributed GQA attention kernel for 8 Trainium2 NeuronCores.

Contract: kernel(**inputs) takes the FULL unsharded inputs of the reference
nn.Module (x, Wq, bq, Wk, bk, Wv, bv, Wo, bo) and returns the FULL
[B, T, E] float32 output.

Sharding: tensor-parallel over kv heads. Core c owns kv head c and q heads
4c..4c+3. Single pass over x: each 512-token chunk is loaded once and
projected to q, k and v back-to-back (softmax scale pre-folded into Wq),
RoPE applied on-chip, v transposed token-major via DMA-xbar. Causal
attention runs in a transposed-score layout (S^T[k,q] so the exp output
feeds the PV matmul with no transpose; a ones-column appended to V yields
softmax denominators; logits are bounded by construction so no
max-subtraction pass). Attention for batch-0 chunk qc starts as soon as
chunk qc is projected. The attention output is resharded head-major ->
token-major with two half-size AllToAlls per batch; o_proj uses the full
Wo on a disjoint 512-token slice. The host only slices/concatenates.
"""

from contextlib import ExitStack

import concourse.bass as bass
import concourse.mybir as mybir
import concourse.tile as tile
from concourse import bacc
from concourse.masks import make_identity

F32 = mybir.dt.float32
BF16 = mybir.dt.bfloat16
AF = mybir.ActivationFunctionType
ALU = mybir.AluOpType

N_CORES = 8


def build(B=2, T=2048, E=2048, D=64, HQ_PER_CORE=4, repeat=1,
          no_collective=False):
    BT = B * T
    DQ = HQ_PER_CORE * D          # 256
    TS = T // N_CORES             # per-core token slice per batch
    KC = E // 128                 # contraction chunks
    NB = BT // 512                # projection column chunks
    QC = T // 512                 # q chunks per batch
    NSPC = 512 // TS              # token slices per 512-col q chunk

    nc = bacc.Bacc("TRN2", target_bir_lowering=False, debug=False,
                   num_devices=N_CORES)

    xT = nc.dram_tensor("xT", [E, BT], BF16, kind="ExternalInput").ap()
    wqT = nc.dram_tensor("wqT", [E, DQ], BF16, kind="ExternalInput").ap()
    wkT = nc.dram_tensor("wkT", [E, D], BF16, kind="ExternalInput").ap()
    wvT = nc.dram_tensor("wvT", [E, D], BF16, kind="ExternalInput").ap()
    bq = nc.dram_tensor("bq", [DQ, 1], F32, kind="ExternalInput").ap()
    bkv = nc.dram_tensor("bkv", [128, 1], F32, kind="ExternalInput").ap()
    woT = nc.dram_tensor("woT", [E, E], BF16, kind="ExternalInput").ap()
    bo = nc.dram_tensor("bo", [1, E], BF16, kind="ExternalInput").ap()
    cos_d = nc.dram_tensor("cosf", [D, BT], BF16, kind="ExternalInput").ap()
    sin_d = nc.dram_tensor("sinm", [D, BT], BF16, kind="ExternalInput").ap()
    mask_d = nc.dram_tensor("mask", [128, 128], BF16, kind="ExternalInput").ap()
    out = nc.dram_tensor("out", [B * TS, E], F32, kind="ExternalOutput").ap()

    args = dict(no_collective=no_collective, B=B, T=T, E=E, D=D,
                HQ=HQ_PER_CORE, BT=BT, DQ=DQ, TS=TS,
                KC=KC, NB=NB, QC=QC, NSPC=NSPC,
                xT=xT, wqT=wqT, wkT=wkT, wvT=wvT, bq=bq, bkv=bkv,
                woT=woT, bo=bo, cos_d=cos_d, sin_d=sin_d, mask_d=mask_d,
                out=out)
    with tile.TileContext(nc) as tc:
        for _ in range(repeat):
            _emit(tc, nc, args)
    nc.compile()
    return nc


def _emit(tc, nc, v):
    B, T, E, D, HQ = v["B"], v["T"], v["E"], v["D"], v["HQ"]
    BT, DQ, TS, KC, NB, QC, NSPC = (v["BT"], v["DQ"], v["TS"], v["KC"],
                                    v["NB"], v["QC"], v["NSPC"])
    xT, wqT, wkT, wvT, bq, bkv, woT, bo = (
        v["xT"], v["wqT"], v["wkT"], v["wvT"], v["bq"], v["bkv"],
        v["woT"], v["bo"])
    cos_d, sin_d, mask_d, out = v["cos_d"], v["sin_d"], v["mask_d"], v["out"]

    with ExitStack() as ctx:
        # ---- persistent SBUF ----
        pers = ctx.enter_context(tc.tile_pool(name="pers", bufs=1))
        wq_sb = pers.tile([128, KC, DQ], BF16, tag="wq")
        wkv_sb = pers.tile([128, KC, 2 * D], BF16, tag="wkv")
        bq_sb = pers.tile([128, DQ // 128], F32, tag="bq")
        bkv_sb = pers.tile([128, 1], F32, tag="bkv")
        bo_sb = pers.tile([1, E], BF16, tag="bo")
        bo_bc = pers.tile([128, E], BF16, tag="bo_bc")
        mask_sb = pers.tile([128, 128], BF16, tag="mask")

        qT_sb = pers.tile([128, HQ // 2, BT], BF16, tag="qT")
        kT_sb = pers.tile([128, BT], BF16, tag="kT")
        vaug_sb = pers.tile([128, BT // 128, D + 1], BF16, tag="vaug")
        cs_sb = pers.tile([128, 2, BT], BF16, tag="cs")  # [:,0]=cos [:,1]=sin
        ones_sb = pers.tile([1, D], BF16, tag="ones")
        ident_sb = pers.tile([128, 128], BF16, tag="ident")
        make_identity(nc, ident_sb[:])

        # only what chunk 0's k/v matmuls need goes first on the DMA queue;
        # the rest is interleaved behind the first x quarters in emit_proj(0)
        nc.sync.dma_start(wkv_sb[:, :, 0:D], wkT.rearrange("(kc p) m -> p kc m", p=128))
        nc.sync.dma_start(wkv_sb[:, :, D:2 * D], wvT.rearrange("(kc p) m -> p kc m", p=128))
        nc.sync.dma_start(bkv_sb[:], bkv[:])
        nc.vector.memset(vaug_sb[:, :, D:D + 1], 1.0)
        nc.vector.memset(ones_sb[:], 1.0)

        dram = ctx.enter_context(tc.tile_pool(name="dram", bufs=1, space="DRAM"))
        nsp = 2
        rows_a2a = 8 * DQ // nsp
        a2a_in = [[dram.tile([rows_a2a, TS], BF16, name=f"a2a_in{b}_{hf}",
                             tag=f"a2a_in{b}_{hf}") for hf in range(nsp)]
                  for b in range(B)]
        a2a_out = [[dram.tile([rows_a2a, TS], BF16, name=f"a2a_out{b}_{hf}",
                              tag=f"a2a_out{b}_{hf}") for hf in range(nsp)]
                   for b in range(B)]

        with ExitStack() as ctx2:
            psb = ctx2.enter_context(tc.tile_pool(name="psb", bufs=2))
            p2sb = ctx2.enter_context(tc.tile_pool(name="p2sb", bufs=3))
            p2ps = ctx2.enter_context(tc.tile_pool(name="p2ps", bufs=2, space="PSUM"))
            p2acc = ctx2.enter_context(tc.tile_pool(name="p2acc", bufs=1, space="PSUM"))
            p3sb = ctx2.enter_context(tc.tile_pool(name="p3sb", bufs=2))
            wop = ctx2.enter_context(tc.tile_pool(name="wop", bufs=2))
            pps_cm = tc.tile_pool(name="pps", bufs=2, space="PSUM")
            pps = pps_cm.__enter__()

            def emit_proj(nb, pending):
                """Load x chunk nb once; project q, k, v; rope; v-transpose."""
                ns = slice(nb * 512, (nb + 1) * 512)
                xt = psb.tile([128, KC, 512], BF16, tag="xt", name="xt")
                kq4 = KC // 4
                for xi in range(4):
                    nc.sync.dma_start(
                        xt[:, xi * kq4:(xi + 1) * kq4],
                        xT[xi * kq4 * 128:(xi + 1) * kq4 * 128, ns]
                        .rearrange("(kc p) n -> p kc n", p=128))
                    if nb == 0 and xi == 0:
                        # cos/sin rows are 64-periodic: load [64, BT] once,
                        # duplicate up; q-proj/attention tables follow later
                        nc.sync.dma_start(cs_sb[0:D, 0], cos_d[:])
                        nc.sync.dma_start(cs_sb[0:D, 1], sin_d[:])
                if nb == 0:
                    nc.sync.dma_start(cs_sb[D:128, 0], cs_sb[0:D, 0])
                    nc.sync.dma_start(cs_sb[D:128, 1], cs_sb[0:D, 1])
                    nc.sync.dma_start(wq_sb[:], wqT.rearrange("(kc p) m -> p kc m", p=128))
                    nc.sync.dma_start(bq_sb[:], bq.rearrange("(mb p) o -> p (mb o)", p=128))
                    nc.sync.dma_start(mask_sb[:], mask_d[:])
                # k|v stacked in one stationary: one N=512 chain computes
                # both (out rows 0:64 = k, 64:128 = v)
                pkv = pps.tile([128, 512], F32, tag="pp", name="pkv")
                for kc in range(KC):
                    nc.tensor.matmul(pkv[:], wkv_sb[:, kc], xt[:, kc],
                                     start=(kc == 0), stop=(kc == KC - 1))
                if pending is not None:
                    # previous head-pair's divide + reshard, emitted behind
                    # this chunk's k/v matmuls so the PE never waits on it
                    pending()
                kvf = psb.tile([128, 512], BF16, tag="kvf", name="kvf")
                nc.vector.tensor_scalar_add(kvf[:], pkv[:], bkv_sb[:])
                # rope on k (rows 0:64)
                ksw = psb.tile([D, 512], BF16, tag="ksw", name="ksw")
                nc.sync.dma_start(ksw[0:32], kvf[32:64])
                nc.sync.dma_start(ksw[32:64], kvf[0:32])
                tk = psb.tile([D, 512], BF16, tag="tk", name="tk")
                nc.vector.tensor_mul(tk[:], kvf[0:D], cs_sb[0:D, 0, ns])
                nc.vector.tensor_mul(ksw[:], ksw[:], cs_sb[0:D, 1, ns])
                nc.vector.tensor_add(kT_sb[0:D, ns], tk[:], ksw[:])
                nc.sync.dma_start(kT_sb[D:128, ns], kT_sb[0:D, ns])
                # v (rows 64:128): transpose token-major on the PE into a
                # bf16 view of a projection-pool PSUM tile, one copy to vaug
                pvt = pps.tile([128, 512], F32, tag="pp", name="pvt")
                pvt_bf = pvt[:].bitcast(BF16)
                for i in range(4):
                    nc.tensor.transpose(pvt_bf[:, i * D:(i + 1) * D],
                                        kvf[D:128, i * 128:(i + 1) * 128],
                                        ident_sb[D:128, D:128])
                # pinned to DVE: an "any" placement could land on the Pool
                # queue and wedge behind an in-flight collective
                nc.vector.tensor_copy(
                    vaug_sb[:, nb * 4:nb * 4 + 4, 0:D],
                    pvt_bf[:, 0:4 * D].rearrange("p (i d) -> p i d", i=4))
                # q: 2 blocks of 128 rows (2 heads each)
                for mb in range(DQ // 128):
                    pq = pps.tile([128, 512], F32, tag="pp", name="pq")
                    for kc in range(KC):
                        nc.tensor.matmul(pq[:], wq_sb[:, kc, mb * 128:(mb + 1) * 128],
                                         xt[:, kc], start=(kc == 0), stop=(kc == KC - 1))
                    qf = psb.tile([128, 512], BF16, tag="qf", name="qf")
                    nc.vector.tensor_scalar_add(qf[:], pq[:], bq_sb[:, mb:mb + 1])
                    qsw = psb.tile([128, 512], BF16, tag="qsw", name="qsw")
                    for g in range(2):
                        o = g * 64
                        nc.sync.dma_start(qsw[o:o + 32], qf[o + 32:o + 64])
                        nc.sync.dma_start(qsw[o + 32:o + 64], qf[o:o + 32])
                    tq = psb.tile([128, 512], BF16, tag="tq", name="tq")
                    nc.vector.tensor_mul(tq[:], qf[:], cs_sb[:, 0, ns])
                    nc.vector.tensor_mul(qsw[:], qsw[:], cs_sb[:, 1, ns])
                    nc.vector.tensor_add(qT_sb[:, mb, ns], tq[:], qsw[:])

            def emit_attention_hp(b, qc, hp, drain_prev):
                n_kb = qc * 4 + 4
                yaccs = None
                for kb in range(n_kb):
                    off = max(0, (kb - 4 * qc) * 128)
                    ncols = 512 - off
                    diag = kb >= 4 * qc
                    qcol = b * T + qc * 512 + off
                    st = p2ps.tile([128, 2, 512], F32, tag="st", name="st")
                    for i in range(2):
                        h = 2 * hp + i
                        po = (h % 2) * D
                        nc.tensor.matmul(
                            st[:, i, :ncols],
                            kT_sb[po:po + D, b * T + kb * 128: b * T + (kb + 1) * 128],
                            qT_sb[po:po + D, h // 2, qcol:qcol + ncols],
                            start=True, stop=True)
                    if kb == 0:
                        # previous head-pair's divide + reshard goes behind
                        # this pair's first scores on the PE queue; its rb
                        # matmuls reuse the yacc PSUM slots, so allocate our
                        # accumulators after it
                        if drain_prev is not None:
                            drain_prev()
                        yaccs = [p2acc.tile([D + 1, 512], F32, name=f"yacc{i}",
                                            tag=f"yacc{i}") for i in range(2)]
                    pt = p2sb.tile([128, 2, 512], BF16, tag="pt", name="pt", bufs=4)
                    nc.scalar.activation(pt[:, :, :ncols], st[:, :, :ncols], AF.Exp)
                    if diag:
                        for i in range(2):
                            nc.vector.tensor_mul(pt[:, i, 0:128], pt[:, i, 0:128],
                                                 mask_sb[:])
                    for i in range(2):
                        nc.tensor.matmul(
                            yaccs[i][:, off:512],
                            vaug_sb[:, b * (T // 128) + kb, :],
                            pt[:, i, :ncols],
                            start=(kb == 0), stop=(kb == n_kb - 1))
                yfs, rs = [], []
                for i in range(2):
                    # one fast copy frees the PSUM accumulator; the divide
                    # chain then runs off-SBUF without stalling the PE.
                    # l-row lands on partition 64 and is moved to partition 0
                    # by DMA (cross-base DVE reads are broken on HW, verified;
                    # DMA is the partition mover).
                    yf = p2sb.tile([D + 1, 512], F32, tag="yf", bufs=2, name="yf")
                    nc.vector.tensor_copy(yf[:], yaccs[i][:])
                    lsb = p2sb.tile([1, 512], F32, tag="lsb", name="lsb")
                    nc.sync.dma_start(lsb[:], yf[D:D + 1, :])
                    r32 = p2sb.tile([1, 512], F32, tag="r32", name="r32")
                    nc.vector.reciprocal_approx_fast(out=r32[:], in_=lsb[:])
                    r = p2sb.tile([1, 512], BF16, tag="r", name="r")
                    nc.vector.tensor_copy(r[:], r32[:])
                    yfs.append(yf)
                    rs.append(r)

                def drain():
                    for i in range(2):
                        h = 2 * hp + i
                        # broadcast r over 64 partitions with a rank-1 matmul
                        # into the freed yacc slot (Pool-queue broadcasts
                        # would serialize behind in-flight collectives)
                        rb = p2acc.tile([D + 1, 512], F32, name=f"rb{i}",
                                        tag=f"yacc{i}")
                        nc.tensor.matmul(rb[0:D, :], ones_sb[:], rs[i][:],
                                         start=True, stop=True)
                        yt = p2sb.tile([D, 512], BF16, tag="yt", bufs=2, name="yt")
                        nc.vector.tensor_mul(yt[:], yfs[i][0:D, :], rb[0:D, :])
                        for ss in range(NSPC):
                            j = qc * NSPC + ss
                            tgt, row = a2a_in[b][h // 2], j * 128 + (h % 2) * D
                            nc.sync.dma_start(tgt[row: row + D, :],
                                              yt[:, ss * TS:(ss + 1) * TS])
                    if qc == QC - 1:
                        emit_a2a(b, hp)
                        load_ya_hf(b, hp)

                return drain

            def emit_a2a(b, hf):
                if v["no_collective"]:
                    nc.sync.dma_start(a2a_out[b][hf][:], a2a_in[b][hf][:])
                else:
                    nc.gpsimd.collective_compute(
                        "AllToAll", ALU.bypass,
                        replica_groups=[list(range(N_CORES))],
                        ins=[a2a_in[b][hf].opt()], outs=[a2a_out[b][hf].opt()])

            yah_store = [[None] * nsp for _ in range(B)]

            def load_ya_hf(b, hf):
                # issued on the gpsimd (SWDGE) queue right behind its
                # collective: the in-order sync queue must never hold a
                # DMA that waits on a collective, or every later x/wo load
                # wedges behind it
                ya = p3sb.tile([128, KC // nsp, TS], BF16, tag=f"ya{b}_{hf}",
                               name=f"ya{b}_{hf}", bufs=1)
                src = a2a_out[b][hf].opt().rearrange("(kc p) t -> p kc t", p=128)
                kq = KC // nsp // 4
                for yi in range(4):
                    nc.gpsimd.dma_start(ya[:, yi * kq:(yi + 1) * kq],
                                        src[:, yi * kq:(yi + 1) * kq])
                yah_store[b][hf] = ya

            woT_r = woT.rearrange("(kc p) m -> p kc m", p=128)

            def emit_oproj(b, p3ps, yah):
                kc_order = ([2 * i for i in range(KC // 2)]
                            + [2 * i + 1 for i in range(KC // 2)])
                for oc in range(E // 512):
                    ocs = slice(oc * 512, (oc + 1) * 512)
                    # Wo streamed per output chunk (each element used once
                    # per batch); bufs=2 prefetches the next chunk
                    wo_t = wop.tile([128, KC, 512], BF16, tag="wo", name="wo")
                    for wi in range(4):
                        nc.sync.dma_start(wo_t[:, wi * 4:(wi + 1) * 4],
                                          woT_r[:, wi * 4:(wi + 1) * 4, ocs])
                    for tb in range((TS + 127) // 128):
                        rows = min(128, TS - tb * 128)
                        po = p3ps.tile([128, 512], F32, tag="po", name="po")
                        for ki, kc in enumerate(kc_order):
                            ya_ap = yah[kc % 2][:, kc // 2]
                            nc.tensor.matmul(po[:rows],
                                             ya_ap[:, tb * 128: tb * 128 + rows],
                                             wo_t[:, kc], start=(ki == 0),
                                             stop=(ki == KC - 1))
                        osb = p3sb.tile([128, 512], F32, tag="osb", name="osb")
                        nc.vector.tensor_tensor(osb[:rows], po[:rows], bo_bc[:rows, ocs],
                                                ALU.add)
                        nc.sync.dma_start(
                            out[b * TS + tb * 128: b * TS + tb * 128 + rows, ocs],
                            osb[:rows])

            pending = None
            for nb in range(NB):
                emit_proj(nb, pending)
                pending = None
                b, qc = divmod(nb, QC)
                for hp in range(HQ // 2):
                    pending = emit_attention_hp(b, qc, hp, pending)
                if nb == QC - 1:
                    nc.sync.dma_start(bo_sb[:], bo[:])
                    nc.gpsimd.partition_broadcast(bo_bc[:], bo_sb[:])
            pending()
            pps_cm.__exit__(None, None, None)
            p3ps = ctx2.enter_context(tc.tile_pool(name="p3ps", bufs=2, space="PSUM"))
            # o_proj b0 placed here so its PE work covers the A2A1 window
            emit_oproj(0, p3ps, yah_store[0])
            emit_oproj(1, p3ps, yah_store[1])


# ---------------------------------------------------------------------------
# host-side sharding, execution, and gather
# ---------------------------------------------------------------------------
import numpy as np
import ml_dtypes

BF = ml_dtypes.bfloat16
ROPE_BASE = 10000.0
_CACHE = {}


def _rope_tables(T, D):
    inv_freq = 1.0 / (ROPE_BASE ** (np.arange(0, D, 2, dtype=np.float64) / D))
    t = np.arange(T, dtype=np.float64)
    freqs = np.einsum("i,j->ij", t, inv_freq)
    return np.cos(freqs), np.sin(freqs)


def _make_core_inputs(x, Wq, bq, Wk, bk, Wv, bv, Wo, bo):
    B, T, E = x.shape
    D = 64
    DQ = Wq.shape[0] // N_CORES
    BT = B * T
    scale = 1.0 / np.sqrt(D)

    xT = np.ascontiguousarray(x.reshape(BT, E).T).astype(BF)
    woT = np.ascontiguousarray(Wo.T).astype(BF)
    bo_row = bo.reshape(1, E).astype(BF)

    cos, sin = _rope_tables(T, D)
    cos32 = np.tile(cos.T, (1, B))
    sin32 = np.tile(sin.T, (1, B))
    cosf = np.tile(cos32, (2, 1)).astype(BF)           # [64, BT]
    sgn = np.where((np.arange(D) % 64) < 32, -1.0, 1.0)[:, None]
    sinm = (np.tile(sin32, (2, 1)) * sgn).astype(BF)   # [64, BT]

    k_idx, q_idx = np.meshgrid(np.arange(128), np.arange(128), indexing="ij")
    mask = (q_idx >= k_idx).astype(BF)

    maps = []
    for c in range(N_CORES):
        qs = slice(c * DQ, (c + 1) * DQ)
        ks = slice(c * D, (c + 1) * D)
        maps.append({
            "xT": xT,
            "wqT": np.ascontiguousarray((Wq[qs] * scale).T).astype(BF),
            "wkT": np.ascontiguousarray(Wk[ks].T).astype(BF),
            "wvT": np.ascontiguousarray(Wv[ks].T).astype(BF),
            "bq": (bq[qs] * scale).reshape(DQ, 1).astype(np.float32),
            "bkv": np.concatenate([bk[ks], bv[ks]]).reshape(128, 1).astype(np.float32),
            "woT": woT,
            "bo": bo_row,
            "cosf": cosf,
            "sinm": sinm,
            "mask": mask,
        })
    return maps


def kernel(x, Wq, bq, Wk, bk, Wv, bv, Wo, bo):
    from concourse import bass_utils

    x = np.asarray(x, dtype=np.float32)
    Wq, bq = np.asarray(Wq, np.float32), np.asarray(bq, np.float32)
    Wk, bk = np.asarray(Wk, np.float32), np.asarray(bk, np.float32)
    Wv, bv = np.asarray(Wv, np.float32), np.asarray(bv, np.float32)
    Wo, bo = np.asarray(Wo, np.float32), np.asarray(bo, np.float32)
    B, T, E = x.shape

    key = (B, T, E)
    if key not in _CACHE:
        _CACHE[key] = build(B=B, T=T, E=E)
    nc = _CACHE[key]

    maps = _make_core_inputs(x, Wq, bq, Wk, bk, Wv, bv, Wo, bo)
    res = bass_utils.run_bass_kernel_spmd(
        nc, maps, core_ids=list(range(N_CORES)))

    TS = T // N_CORES
    full = np.empty((B, T, E), dtype=np.float32)
    for c in range(N_CORES):
        o = res.results[c]["out"]
        for b in range(B):
            full[b, c * TS:(c + 1) * TS] = o[b * TS:(b + 1) * TS]
    return full


# revision 33
# speedup vs baseline: 1.0578x; 1.0393x over previous
"""Distributed GQA attention kernel for 8 Trainium2 NeuronCores.

Contract: kernel(**inputs) takes the FULL unsharded inputs of the reference
nn.Module (x, Wq, bq, Wk, bk, Wv, bv, Wo, bo) and returns the FULL
[B, T, E] float32 output.

Sharding: tensor-parallel over kv heads. Core c owns kv head c and q heads
4c..4c+3. Single pass over x: each 512-token chunk is loaded once and
projected to q, k and v back-to-back (softmax scale pre-folded into Wq),
RoPE applied on-chip, v transposed token-major on the PE. Causal attention
runs in a transposed-score layout (S^T[k,q] so the exp output feeds the PV
matmul with no transpose; the two heads of a pair run concurrently on PE
row-tiles since the contraction is only 64 deep; a ones-column appended to
V yields softmax denominators; logits are bounded by construction so no
max-subtraction pass). Attention for batch-0 chunk qc starts as soon as
chunk qc is projected. The softmax reciprocal is broadcast with a rank-1
matmul instead of a gpsimd partition_broadcast so the Pool queue carries
only collectives (anything else wedges behind an in-flight AllToAll); for
the same reason the a2a-output loads issue from the gpsimd SWDGE queue,
never the sync queue. The attention output is resharded head-major ->
token-major with two half-size AllToAlls per batch, fired per head pair as
soon as its last chunk drains; o_proj streams Wo per output chunk and uses
an even-kc-first accumulation order so it can start with only the first
half-collective landed. The host only slices/concatenates.
"""

from contextlib import ExitStack

import concourse.bass as bass
import concourse.mybir as mybir
import concourse.tile as tile
from concourse import bacc
from concourse.masks import make_identity

F32 = mybir.dt.float32
BF16 = mybir.dt.bfloat16
AF = mybir.ActivationFunctionType
ALU = mybir.AluOpType

N_CORES = 8


def build(B=2, T=2048, E=2048, D=64, HQ_PER_CORE=4, repeat=1,
          no_collective=False):
    BT = B * T
    DQ = HQ_PER_CORE * D          # 256
    TS = T // N_CORES             # per-core token slice per batch
    KC = E // 128                 # contraction chunks
    NB = BT // 512                # projection column chunks
    QC = T // 512                 # q chunks per batch
    NSPC = 512 // TS              # token slices per 512-col q chunk

    nc = bacc.Bacc("TRN2", target_bir_lowering=False, debug=False,
                   num_devices=N_CORES)

    xT = nc.dram_tensor("xT", [E, BT], BF16, kind="ExternalInput").ap()
    wqT = nc.dram_tensor("wqT", [E, DQ], BF16, kind="ExternalInput").ap()
    wkT = nc.dram_tensor("wkT", [E, D], BF16, kind="ExternalInput").ap()
    wvT = nc.dram_tensor("wvT", [E, D], BF16, kind="ExternalInput").ap()
    bq = nc.dram_tensor("bq", [DQ, 1], F32, kind="ExternalInput").ap()
    bkv = nc.dram_tensor("bkv", [128, 1], F32, kind="ExternalInput").ap()
    woT = nc.dram_tensor("woT", [E, E], BF16, kind="ExternalInput").ap()
    bo = nc.dram_tensor("bo", [1, E], BF16, kind="ExternalInput").ap()
    cos_d = nc.dram_tensor("cosf", [D, BT], BF16, kind="ExternalInput").ap()
    sin_d = nc.dram_tensor("sinm", [D, BT], BF16, kind="ExternalInput").ap()
    mask_d = nc.dram_tensor("mask", [128, 128], BF16, kind="ExternalInput").ap()
    out = nc.dram_tensor("out", [B * TS, E], F32, kind="ExternalOutput").ap()

    args = dict(no_collective=no_collective, B=B, T=T, E=E, D=D,
                HQ=HQ_PER_CORE, BT=BT, DQ=DQ, TS=TS,
                KC=KC, NB=NB, QC=QC, NSPC=NSPC,
                xT=xT, wqT=wqT, wkT=wkT, wvT=wvT, bq=bq, bkv=bkv,
                woT=woT, bo=bo, cos_d=cos_d, sin_d=sin_d, mask_d=mask_d,
                out=out)
    with tile.TileContext(nc) as tc:
        for _ in range(repeat):
            _emit(tc, nc, args)
    nc.compile()
    return nc


def _emit(tc, nc, v):
    B, T, E, D, HQ = v["B"], v["T"], v["E"], v["D"], v["HQ"]
    BT, DQ, TS, KC, NB, QC, NSPC = (v["BT"], v["DQ"], v["TS"], v["KC"],
                                    v["NB"], v["QC"], v["NSPC"])
    xT, wqT, wkT, wvT, bq, bkv, woT, bo = (
        v["xT"], v["wqT"], v["wkT"], v["wvT"], v["bq"], v["bkv"],
        v["woT"], v["bo"])
    cos_d, sin_d, mask_d, out = v["cos_d"], v["sin_d"], v["mask_d"], v["out"]

    with ExitStack() as ctx:
        # ---- persistent SBUF ----
        pers = ctx.enter_context(tc.tile_pool(name="pers", bufs=1))
        wq_sb = pers.tile([128, KC, DQ], BF16, tag="wq")
        wkv_sb = pers.tile([128, KC, 2 * D], BF16, tag="wkv")
        bq_sb = pers.tile([128, DQ // 128], F32, tag="bq")
        bkv_sb = pers.tile([128, 1], F32, tag="bkv")
        bo_sb = pers.tile([1, E], BF16, tag="bo")
        bo_bc = pers.tile([128, E], BF16, tag="bo_bc")
        mask_sb = pers.tile([128, 128], BF16, tag="mask")

        qT_sb = pers.tile([128, HQ // 2, BT], BF16, tag="qT")
        kT_sb = pers.tile([128, BT], BF16, tag="kT")
        vaug_sb = pers.tile([128, BT // 128, D + 1], BF16, tag="vaug")
        cs_sb = pers.tile([128, 2, BT], BF16, tag="cs")  # [:,0]=cos [:,1]=sin
        ones_sb = pers.tile([1, D], BF16, tag="ones")
        ident_sb = pers.tile([128, 128], BF16, tag="ident")
        make_identity(nc, ident_sb[:])

        # only what chunk 0's k/v matmuls need goes first on the DMA queue;
        # the rest is interleaved behind the first x quarters in emit_proj(0)
        nc.sync.dma_start(wkv_sb[:, :, 0:D], wkT.rearrange("(kc p) m -> p kc m", p=128))
        nc.sync.dma_start(wkv_sb[:, :, D:2 * D], wvT.rearrange("(kc p) m -> p kc m", p=128))
        nc.sync.dma_start(bkv_sb[:], bkv[:])
        nc.vector.memset(vaug_sb[:, :, D:D + 1], 1.0)
        nc.vector.memset(ones_sb[:], 1.0)

        dram = ctx.enter_context(tc.tile_pool(name="dram", bufs=1, space="DRAM"))
        nsp = 2
        rows_a2a = 8 * DQ // nsp
        a2a_in = [[dram.tile([rows_a2a, TS], BF16, name=f"a2a_in{b}_{hf}",
                             tag=f"a2a_in{b}_{hf}") for hf in range(nsp)]
                  for b in range(B)]
        a2a_out = [[dram.tile([rows_a2a, TS], BF16, name=f"a2a_out{b}_{hf}",
                              tag=f"a2a_out{b}_{hf}") for hf in range(nsp)]
                   for b in range(B)]

        with ExitStack() as ctx2:
            psb = ctx2.enter_context(tc.tile_pool(name="psb", bufs=2))
            p2sb = ctx2.enter_context(tc.tile_pool(name="p2sb", bufs=3))
            p2ps = ctx2.enter_context(tc.tile_pool(name="p2ps", bufs=2, space="PSUM"))
            p2acc = ctx2.enter_context(tc.tile_pool(name="p2acc", bufs=1, space="PSUM"))
            p3sb = ctx2.enter_context(tc.tile_pool(name="p3sb", bufs=2))
            wop = ctx2.enter_context(tc.tile_pool(name="wop", bufs=2))
            pps_cm = tc.tile_pool(name="pps", bufs=2, space="PSUM")
            pps = pps_cm.__enter__()

            def emit_proj(nb, pending):
                """Load x chunk nb once; project q, k, v; rope; v-transpose."""
                ns = slice(nb * 512, (nb + 1) * 512)
                xt = psb.tile([128, KC, 512], BF16, tag="xt", name="xt")
                kq4 = KC // 4
                for xi in range(4):
                    nc.sync.dma_start(
                        xt[:, xi * kq4:(xi + 1) * kq4],
                        xT[xi * kq4 * 128:(xi + 1) * kq4 * 128, ns]
                        .rearrange("(kc p) n -> p kc n", p=128))
                    if nb == 0 and xi == 0:
                        # cos/sin rows are 64-periodic: load [64, BT] once,
                        # duplicate up; q-proj/attention tables follow later
                        nc.sync.dma_start(cs_sb[0:D, 0], cos_d[:])
                        nc.sync.dma_start(cs_sb[0:D, 1], sin_d[:])
                if nb == 0:
                    nc.sync.dma_start(cs_sb[D:128, 0], cs_sb[0:D, 0])
                    nc.sync.dma_start(cs_sb[D:128, 1], cs_sb[0:D, 1])
                    nc.sync.dma_start(wq_sb[:], wqT.rearrange("(kc p) m -> p kc m", p=128))
                    nc.sync.dma_start(bq_sb[:], bq.rearrange("(mb p) o -> p (mb o)", p=128))
                    nc.sync.dma_start(mask_sb[:], mask_d[:])
                # k|v stacked in one stationary: one N=512 chain computes
                # both (out rows 0:64 = k, 64:128 = v)
                pkv = pps.tile([128, 512], F32, tag="pp", name="pkv")
                for kc in range(KC):
                    nc.tensor.matmul(pkv[:], wkv_sb[:, kc], xt[:, kc],
                                     start=(kc == 0), stop=(kc == KC - 1))
                if pending is not None:
                    # previous head-pair's divide + reshard, emitted behind
                    # this chunk's k/v matmuls so the PE never waits on it
                    pending()
                kvf = psb.tile([128, 512], BF16, tag="kvf", name="kvf")
                nc.vector.tensor_scalar_add(kvf[:], pkv[:], bkv_sb[:])
                # rope on k (rows 0:64)
                ksw = psb.tile([D, 512], BF16, tag="ksw", name="ksw")
                nc.sync.dma_start(ksw[0:32], kvf[32:64])
                nc.sync.dma_start(ksw[32:64], kvf[0:32])
                tk = psb.tile([D, 512], BF16, tag="tk", name="tk")
                nc.vector.tensor_mul(tk[:], kvf[0:D], cs_sb[0:D, 0, ns])
                nc.vector.tensor_mul(ksw[:], ksw[:], cs_sb[0:D, 1, ns])
                nc.vector.tensor_add(kT_sb[0:D, ns], tk[:], ksw[:])
                nc.sync.dma_start(kT_sb[D:128, ns], kT_sb[0:D, ns])
                # v (rows 64:128): transpose token-major on the PE into a
                # bf16 view of a projection-pool PSUM tile, one copy to vaug
                pvt = pps.tile([128, 512], F32, tag="pp", name="pvt")
                pvt_bf = pvt[:].bitcast(BF16)
                for i in range(4):
                    nc.tensor.transpose(pvt_bf[:, i * D:(i + 1) * D],
                                        kvf[D:128, i * 128:(i + 1) * 128],
                                        ident_sb[D:128, D:128])
                # pinned to DVE (what the scheduler picks for "any" here):
                # a Pool placement would wedge behind an in-flight collective
                nc.vector.tensor_copy(
                    vaug_sb[:, nb * 4:nb * 4 + 4, 0:D],
                    pvt_bf[:, 0:4 * D].rearrange("p (i d) -> p i d", i=4))
                # q: 2 blocks of 128 rows (2 heads each)
                for mb in range(DQ // 128):
                    pq = pps.tile([128, 512], F32, tag="pp", name="pq")
                    for kc in range(KC):
                        nc.tensor.matmul(pq[:], wq_sb[:, kc, mb * 128:(mb + 1) * 128],
                                         xt[:, kc], start=(kc == 0), stop=(kc == KC - 1))
                    qf = psb.tile([128, 512], BF16, tag="qf", name="qf")
                    nc.vector.tensor_scalar_add(qf[:], pq[:], bq_sb[:, mb:mb + 1])
                    qsw = psb.tile([128, 512], BF16, tag="qsw", name="qsw")
                    for g in range(2):
                        o = g * 64
                        nc.sync.dma_start(qsw[o:o + 32], qf[o + 32:o + 64])
                        nc.sync.dma_start(qsw[o + 32:o + 64], qf[o:o + 32])
                    tq = psb.tile([128, 512], BF16, tag="tq", name="tq")
                    nc.vector.tensor_mul(tq[:], qf[:], cs_sb[:, 0, ns])
                    nc.vector.tensor_mul(qsw[:], qsw[:], cs_sb[:, 1, ns])
                    nc.vector.tensor_add(qT_sb[:, mb, ns], tq[:], qsw[:])

            def emit_attention_hp(b, qc, hp, drain_prev):
                n_kb = qc * 4 + 4
                yaccs = None
                for kb in range(n_kb):
                    off = max(0, (kb - 4 * qc) * 128)
                    ncols = 512 - off
                    diag = kb >= 4 * qc
                    qcol = b * T + qc * 512 + off
                    st = p2ps.tile([128, 2, 512], F32, tag="st", name="st")
                    for i in range(2):
                        h = 2 * hp + i
                        po = (h % 2) * D
                        nc.tensor.matmul(
                            st[:, i, :ncols],
                            kT_sb[po:po + D, b * T + kb * 128: b * T + (kb + 1) * 128],
                            qT_sb[po:po + D, h // 2, qcol:qcol + ncols],
                            start=True, stop=True)
                    if kb == 0:
                        # previous head-pair's divide + reshard goes behind
                        # this pair's first scores on the PE queue; its rb
                        # matmuls reuse the yacc PSUM slots, so allocate our
                        # accumulators after it
                        if drain_prev is not None:
                            drain_prev()
                        yaccs = [p2acc.tile([D + 1, 512], F32, name=f"yacc{i}",
                                            tag=f"yacc{i}") for i in range(2)]
                    pt = p2sb.tile([128, 2, 512], BF16, tag="pt", name="pt", bufs=4)
                    nc.scalar.activation(pt[:, :, :ncols], st[:, :, :ncols], AF.Exp)
                    if diag:
                        for i in range(2):
                            nc.vector.tensor_mul(pt[:, i, 0:128], pt[:, i, 0:128],
                                                 mask_sb[:])
                    for i in range(2):
                        nc.tensor.matmul(
                            yaccs[i][:, off:512],
                            vaug_sb[:, b * (T // 128) + kb, :],
                            pt[:, i, :ncols],
                            start=(kb == 0), stop=(kb == n_kb - 1))
                yfs, rs = [], []
                for i in range(2):
                    # one fast copy frees the PSUM accumulator; the divide
                    # chain then runs off-SBUF without stalling the PE.
                    # l-row lands on partition 64 and is moved to partition 0
                    # by DMA (cross-base DVE reads are broken on HW, verified;
                    # DMA is the partition mover).
                    yf = p2sb.tile([D + 1, 512], F32, tag="yf", bufs=2, name="yf")
                    nc.vector.tensor_copy(yf[:], yaccs[i][:])
                    lsb = p2sb.tile([1, 512], F32, tag="lsb", name="lsb")
                    nc.sync.dma_start(lsb[:], yf[D:D + 1, :])
                    r32 = p2sb.tile([1, 512], F32, tag="r32", name="r32")
                    nc.vector.reciprocal_approx_fast(out=r32[:], in_=lsb[:])
                    r = p2sb.tile([1, 512], BF16, tag="r", name="r")
                    nc.vector.tensor_copy(r[:], r32[:])
                    yfs.append(yf)
                    rs.append(r)

                def drain():
                    for i in range(2):
                        h = 2 * hp + i
                        # broadcast r over 64 partitions with a rank-1 matmul
                        # into the freed yacc slot (Pool-queue broadcasts
                        # would serialize behind in-flight collectives)
                        rb = p2acc.tile([D + 1, 512], F32, name=f"rb{i}",
                                        tag=f"yacc{i}")
                        nc.tensor.matmul(rb[0:D, :], ones_sb[:], rs[i][:],
                                         start=True, stop=True)
                        yt = p2sb.tile([D, 512], BF16, tag="yt", bufs=2, name="yt")
                        nc.vector.tensor_mul(yt[:], yfs[i][0:D, :], rb[0:D, :])
                        for ss in range(NSPC):
                            j = qc * NSPC + ss
                            tgt, row = a2a_in[b][h // 2], j * 128 + (h % 2) * D
                            nc.sync.dma_start(tgt[row: row + D, :],
                                              yt[:, ss * TS:(ss + 1) * TS])
                    if qc == QC - 1:
                        emit_a2a(b, hp)
                        load_ya_hf(b, hp)

                return drain

            def emit_a2a(b, hf):
                if v["no_collective"]:
                    nc.sync.dma_start(a2a_out[b][hf][:], a2a_in[b][hf][:])
                else:
                    nc.gpsimd.collective_compute(
                        "AllToAll", ALU.bypass,
                        replica_groups=[list(range(N_CORES))],
                        ins=[a2a_in[b][hf].opt()], outs=[a2a_out[b][hf].opt()])

            yah_store = [[None] * nsp for _ in range(B)]

            def load_ya_hf(b, hf):
                # issued on the gpsimd (SWDGE) queue right behind its
                # collective: the in-order sync queue must never hold a
                # DMA that waits on a collective, or every later x/wo load
                # wedges behind it
                ya = p3sb.tile([128, KC // nsp, TS], BF16, tag=f"ya{b}_{hf}",
                               name=f"ya{b}_{hf}", bufs=1)
                src = a2a_out[b][hf].opt().rearrange("(kc p) t -> p kc t", p=128)
                kq = KC // nsp // 4
                for yi in range(4):
                    nc.gpsimd.dma_start(ya[:, yi * kq:(yi + 1) * kq],
                                        src[:, yi * kq:(yi + 1) * kq])
                yah_store[b][hf] = ya

            woT_r = woT.rearrange("(kc p) m -> p kc m", p=128)

            def emit_oproj(b, p3ps, yah):
                kc_order = ([2 * i for i in range(KC // 2)]
                            + [2 * i + 1 for i in range(KC // 2)])
                for oc in range(E // 512):
                    ocs = slice(oc * 512, (oc + 1) * 512)
                    # Wo streamed per output chunk (each element used once
                    # per batch); bufs=2 prefetches the next chunk
                    wo_t = wop.tile([128, KC, 512], BF16, tag="wo", name="wo")
                    for wi in range(4):
                        nc.sync.dma_start(wo_t[:, wi * 4:(wi + 1) * 4],
                                          woT_r[:, wi * 4:(wi + 1) * 4, ocs])
                    for tb in range((TS + 127) // 128):
                        rows = min(128, TS - tb * 128)
                        po = p3ps.tile([128, 512], F32, tag="po", name="po")
                        for ki, kc in enumerate(kc_order):
                            ya_ap = yah[kc % 2][:, kc // 2]
                            nc.tensor.matmul(po[:rows],
                                             ya_ap[:, tb * 128: tb * 128 + rows],
                                             wo_t[:, kc], start=(ki == 0),
                                             stop=(ki == KC - 1))
                        osb = p3sb.tile([128, 512], F32, tag="osb", name="osb")
                        nc.vector.tensor_tensor(osb[:rows], po[:rows], bo_bc[:rows, ocs],
                                                ALU.add)
                        nc.sync.dma_start(
                            out[b * TS + tb * 128: b * TS + tb * 128 + rows, ocs],
                            osb[:rows])

            pending = None
            for nb in range(NB):
                emit_proj(nb, pending)
                pending = None
                b, qc = divmod(nb, QC)
                for hp in range(HQ // 2):
                    pending = emit_attention_hp(b, qc, hp, pending)
                if nb == QC - 1:
                    nc.sync.dma_start(bo_sb[:], bo[:])
                    nc.gpsimd.partition_broadcast(bo_bc[:], bo_sb[:])
            pending()
            pps_cm.__exit__(None, None, None)
            p3ps = ctx2.enter_context(tc.tile_pool(name="p3ps", bufs=2, space="PSUM"))
            # o_proj b0 placed here so its PE work covers the A2A1 window
            emit_oproj(0, p3ps, yah_store[0])
            emit_oproj(1, p3ps, yah_store[1])


# ---------------------------------------------------------------------------
# host-side sharding, execution, and gather
# ---------------------------------------------------------------------------
import numpy as np
import ml_dtypes

BF = ml_dtypes.bfloat16
ROPE_BASE = 10000.0
_CACHE = {}


def _rope_tables(T, D):
    inv_freq = 1.0 / (ROPE_BASE ** (np.arange(0, D, 2, dtype=np.float64) / D))
    t = np.arange(T, dtype=np.float64)
    freqs = np.einsum("i,j->ij", t, inv_freq)
    return np.cos(freqs), np.sin(freqs)


def _make_core_inputs(x, Wq, bq, Wk, bk, Wv, bv, Wo, bo):
    B, T, E = x.shape
    D = 64
    DQ = Wq.shape[0] // N_CORES
    BT = B * T
    scale = 1.0 / np.sqrt(D)

    xT = np.ascontiguousarray(x.reshape(BT, E).T).astype(BF)
    woT = np.ascontiguousarray(Wo.T).astype(BF)
    bo_row = bo.reshape(1, E).astype(BF)

    cos, sin = _rope_tables(T, D)
    cos32 = np.tile(cos.T, (1, B))
    sin32 = np.tile(sin.T, (1, B))
    cosf = np.tile(cos32, (2, 1)).astype(BF)           # [64, BT]
    sgn = np.where((np.arange(D) % 64) < 32, -1.0, 1.0)[:, None]
    sinm = (np.tile(sin32, (2, 1)) * sgn).astype(BF)   # [64, BT]

    k_idx, q_idx = np.meshgrid(np.arange(128), np.arange(128), indexing="ij")
    mask = (q_idx >= k_idx).astype(BF)

    maps = []
    for c in range(N_CORES):
        qs = slice(c * DQ, (c + 1) * DQ)
        ks = slice(c * D, (c + 1) * D)
        maps.append({
            "xT": xT,
            "wqT": np.ascontiguousarray((Wq[qs] * scale).T).astype(BF),
            "wkT": np.ascontiguousarray(Wk[ks].T).astype(BF),
            "wvT": np.ascontiguousarray(Wv[ks].T).astype(BF),
            "bq": (bq[qs] * scale).reshape(DQ, 1).astype(np.float32),
            "bkv": np.concatenate([bk[ks], bv[ks]]).reshape(128, 1).astype(np.float32),
            "woT": woT,
            "bo": bo_row,
            "cosf": cosf,
            "sinm": sinm,
            "mask": mask,
        })
    return maps


def kernel(x, Wq, bq, Wk, bk, Wv, bv, Wo, bo):
    from concourse import bass_utils

    x = np.asarray(x, dtype=np.float32)
    Wq, bq = np.asarray(Wq, np.float32), np.asarray(bq, np.float32)
    Wk, bk = np.asarray(Wk, np.float32), np.asarray(bk, np.float32)
    Wv, bv = np.asarray(Wv, np.float32), np.asarray(bv, np.float32)
    Wo, bo = np.asarray(Wo, np.float32), np.asarray(bo, np.float32)
    B, T, E = x.shape

    key = (B, T, E)
    if key not in _CACHE:
        _CACHE[key] = build(B=B, T=T, E=E)
    nc = _CACHE[key]

    maps = _make_core_inputs(x, Wq, bq, Wk, bk, Wv, bv, Wo, bo)
    res = bass_utils.run_bass_kernel_spmd(
        nc, maps, core_ids=list(range(N_CORES)))

    TS = T // N_CORES
    full = np.empty((B, T, E), dtype=np.float32)
    for c in range(N_CORES):
        o = res.results[c]["out"]
        for b in range(B):
            full[b, c * TS:(c + 1) * TS] = o[b * TS:(b + 1) * TS]
    return full
